# revision 5
# baseline (speedup 1.0000x reference)
import sys

sys.path.insert(0, "/opt/trn_rl_repo")

import numpy as np

from concourse import bass, mybir, tile
from concourse import bass_utils
from concourse.masks import make_identity

B, N, K, D = 4, 16384, 32, 64
HALF = 8192            # points per core
PP = HALF // 2         # 4096 point-pairs per core
M2 = PP * K            # 131072 columns (2 points per column)
PCH = 512              # point-pairs per chunk
NCHUNK = PP // PCH     # 8
G = 512                # columns per group (one PSUM bank)
GPC = K * PCH // G     # 32 groups per chunk (one per k-slab)
NG = NCHUNK * GPC      # 256 groups total
XGB = 16               # groups per xg DMA block (8192 cols, 2 MiB)

TRACE = False
LAST_RESULTS = None
_BUILT = None


def _build():
    f32 = mybir.dt.float32
    f16 = mybir.dt.float16
    Copy = mybir.ActivationFunctionType.Copy
    Prelu = mybir.ActivationFunctionType.Prelu
    mult = mybir.AluOpType.mult

    nc = bass.Bass()
    xg2_d = nc.declare_dram_parameter("xg2", [128, M2], f16, False)
    relb_d = nc.declare_dram_parameter("relb", [NCHUNK, 4, 8, 8 * PCH], f16, False)
    W1bq_d = nc.declare_dram_parameter("W1bq", [128, 128], f16, False)
    W2d_d = nc.declare_dram_parameter("W2d", [128, 128], f16, False)
    out_d = nc.declare_dram_parameter("out", [128, PP], f16, True)

    with tile.TileContext(nc) as tc:
        frees = []

        def T(shape, dtype, name):
            t, f = tc.tile(shape, dtype, name=name)
            frees.append(f)
            return t

        W1bq_sb = T([128, 128], f16, "W1bq_sb")
        W2d_sb = T([128, 128], f16, "W2d_sb")
        ident = T([128, 128], f16, "ident")
        out_sb = T([128, PP], f16, "out_sb")

        nc.sync.dma_start(W1bq_sb[:, :], W1bq_d[:, :])
        nc.sync.dma_start(W2d_sb[:, :], W2d_d[:, :])
        make_identity(nc, ident[:, :])

        with tc.tile_pool(name="relbpool", bufs=2) as rbpl, \
             tc.tile_pool(name="xgpool", bufs=4) as xgpl, \
             tc.tile_pool(name="upool", bufs=2, space="PSUM") as upl, \
             tc.tile_pool(name="wpool", bufs=3, space="PSUM") as wpl, \
             tc.tile_pool(name="accpool", bufs=1, space="PSUM") as accpl, \
             tc.tile_pool(name="rspool", bufs=3) as rspl, \
             tc.tile_pool(name="tpool", bufs=5) as tpl:

            us, rss, ws, ts, accs = {}, {}, {}, {}, {}
            xgs = {}

            def relb_load(q):
                relb_t = rbpl.tile([128, 8 * PCH], f16, name="relb")
                for r in range(4):
                    nc.sync.dma_start(relb_t[32 * r:32 * r + 8, :],
                                      relb_d[q, r, :, :])
                return relb_t

            relbs = {0: relb_load(0)}

            def xg_load(blk):
                xg_t = xgpl.tile([128, XGB * G], f16, name="xg")
                nc.sync.dma_start(xg_t[:, :],
                                  xg2_d[:, blk * XGB * G:(blk + 1) * XGB * G])
                return xg_t

            xgs[0] = xg_load(0)

            # 5-stage software pipeline over 256 groups, Prelu per PAIR:
            # S0 mm1(g) into u-pair half | S1 prelu(pair m at s=2m+2, FD=1024)
            # S2 mm2(g-3) | S3 mult(g-4) | S4 mm3(g-5)
            for s in range(NG + 5):
                g = s
                if g < NG:
                    q, gc = g // GPC, g % GPC
                    k = gc
                    r = k // 8
                    if g % XGB == 0 and g + XGB < NG:
                        xgs[g // XGB + 1] = xg_load(g // XGB + 1)
                    if gc == 0 and q + 1 < NCHUNK:
                        relbs[q + 1] = relb_load(q + 1)
                    if gc == 0:
                        accs[q] = accpl.tile([128, PCH], f32, name="acc")
                    if g % 2 == 0:
                        us[g // 2] = upl.tile([128, 2 * G], f32, name="u")
                    u = us[g // 2]
                    lo = (k % 8) * PCH
                    nc.tensor.matmul(u[:, (g % 2) * G:(g % 2) * G + G],
                                     lhsT=W1bq_sb[32 * r:32 * r + 8, :],
                                     rhs=relbs[q][32 * r:32 * r + 8,
                                                  lo:lo + G],
                                     start=True, stop=True,
                                     tile_position=(32 * r, 0))
                if s % 2 == 0 and s >= 2:
                    m1 = (s - 2) // 2
                    if m1 < NG // 2:
                        rs = rspl.tile([128, 2 * G], f16, name="rs")
                        rss[m1] = rs
                        nc.scalar.activation(rs[:, :], us.pop(m1)[:, :],
                                             Prelu, alpha=0.1)
                g2 = s - 3
                if 0 <= g2 < NG:
                    w = wpl.tile([128, G], f32, name="w")
                    ws[g2] = w
                    rs2 = rss[g2 // 2]
                    nc.tensor.matmul(w[:, :], lhsT=W2d_sb[:, :],
                                     rhs=rs2[:, (g2 % 2) * G:
                                             (g2 % 2) * G + G],
                                     start=True, stop=True)
                    if g2 % 2 == 1:
                        rss.pop(g2 // 2)
                g3 = s - 4
                if 0 <= g3 < NG:
                    t = tpl.tile([128, G], f16, name="t")
                    ts[g3] = t
                    xg_t = xgs[g3 // XGB]
                    xlo = (g3 % XGB) * G
                    nc.vector.tensor_tensor(t[:, :], ws.pop(g3)[:, :],
                                            xg_t[:, xlo:xlo + G], mult)
                g4 = s - 5
                if 0 <= g4 < NG:
                    q4, gc4 = g4 // GPC, g4 % GPC
                    k4 = gc4
                    nc.tensor.matmul(accs[q4][:, :],
                                     lhsT=ident[:, :], rhs=ts.pop(g4)[:, :],
                                     start=(k4 == 0), stop=(k4 == K - 1))
                    if gc4 == GPC - 1:
                        nc.vector.tensor_copy(
                            out_sb[:, q4 * PCH:(q4 + 1) * PCH],
                            accs.pop(q4)[:, :])
                        nc.sync.dma_start(
                            out_d[:, q4 * PCH:(q4 + 1) * PCH],
                            out_sb[:, q4 * PCH:(q4 + 1) * PCH])
        for f in reversed(frees):
            f()

    import bass_rust
    bass_rust.move_matmul_waits_to_ldweights(nc.m)
    bass_rust.generate_event_semaphores(nc)
    mybir.codegen_inst_isa_subclasses(nc)
    return nc


def _get_nc():
    global _BUILT
    if _BUILT is None:
        _BUILT = _build()
    return _BUILT


def _prep_core(x16, pos, nidx, c, W1bq, W2d):
    b, hh = c // 2, c % 2
    sl = slice(hh * HALF, (hh + 1) * HALF)
    idxh = nidx[b, sl]                                  # [HALF, K]
    xg = x16[b][idxh]                                   # [HALF, K, 64] f16
    rel = (pos[b, sl][:, None, :] - pos[b][idxh]).astype(np.float16)

    xgA = xg[0::2].reshape(NCHUNK, PCH, K, D).transpose(0, 2, 1, 3)
    xgB = xg[1::2].reshape(NCHUNK, PCH, K, D).transpose(0, 2, 1, 3)
    xg2 = np.empty((128, M2), np.float16)
    xg2[0:64] = xgA.reshape(M2, D).T
    xg2[64:128] = xgB.reshape(M2, D).T

    rb = np.empty((8, M2), np.float16)
    relA = rel[0::2].reshape(NCHUNK, PCH, K, 3).transpose(0, 2, 1, 3)
    relB = rel[1::2].reshape(NCHUNK, PCH, K, 3).transpose(0, 2, 1, 3)
    rb[0:3] = relA.reshape(M2, 3).T
    rb[3] = 1.0
    rb[4:7] = relB.reshape(M2, 3).T
    rb[7] = 1.0
    relb = np.ascontiguousarray(
        rb.reshape(8, NCHUNK, 4, 8 * PCH).transpose(1, 2, 0, 3))
    return dict(xg2=np.ascontiguousarray(xg2), relb=relb,
                W1bq=W1bq, W2d=W2d)


def kernel(x, pos, neighbor_idx, W1, b1, W2, b2):
    nc = _get_nc()
    W1b = np.vstack([W1, b1[None, :]]).astype(np.float16)   # [4, 64]
    W1bq = np.zeros((128, 128), np.float16)
    for r in range(4):
        W1bq[32 * r:32 * r + 4, 0:64] = W1b
        W1bq[32 * r + 4:32 * r + 8, 64:128] = W1b
    W2d = np.zeros((128, 128), np.float16)
    W2f = W2.astype(np.float16)
    W2d[0:64, 0:64] = W2f
    W2d[64:128, 64:128] = W2f

    x16 = x.astype(np.float16)
    in_maps = [_prep_core(x16, pos, neighbor_idx, c, W1bq, W2d)
               for c in range(8)]
    global LAST_RESULTS
    res = bass_utils.run_bass_kernel_spmd(nc, in_maps, list(range(8)),
                                          trace=TRACE)
    LAST_RESULTS = res
    out = np.empty((B, N, D), np.float32)
    for c in range(8):
        b, hh = c // 2, c % 2
        r = np.asarray(res.results[c]["out"])               # [128, PP] f16
        half = r.T.reshape(PP, 2, D).reshape(HALF, D)
        out[b, hh * HALF:(hh + 1) * HALF] = half.astype(np.float32)
    if np.any(b2):
        for b in range(B):
            s = x[b][neighbor_idx[b]].sum(axis=1)
            out[b] += b2[None, :] * s
    return out


# revision 7
# speedup vs baseline: 1.1536x; 1.1536x over previous
import sys

sys.path.insert(0, "/opt/trn_rl_repo")

import numpy as np

from concourse import bass, mybir, tile
from concourse import bass_utils
from concourse.masks import make_identity

B, N, K, D = 4, 16384, 32, 64
HALF = 8192            # points per core
PP = HALF // 2         # 4096 point-pairs per core
M2 = PP * K            # 131072 columns (2 points per column)
PCH = 1024             # point-pairs per chunk
NCHUNK = PP // PCH     # 4
G = 512                # columns per group (one PSUM bank)
GPC = K * PCH // G     # 64 groups per chunk
NG = NCHUNK * GPC      # 256 groups total
XGB = 16               # groups per xg DMA block (8192 cols, 2 MiB)

TRACE = False
LAST_RESULTS = None
_BUILT = None


def _build():
    f32 = mybir.dt.float32
    f16 = mybir.dt.float16
    Copy = mybir.ActivationFunctionType.Copy
    Prelu = mybir.ActivationFunctionType.Prelu
    mult = mybir.AluOpType.mult

    nc = bass.Bass()
    xg2_d = nc.declare_dram_parameter("xg2", [128, M2], f16, False)
    relb_d = nc.declare_dram_parameter("relb", [NCHUNK, 4, 8, 8 * PCH], f16, False)
    W1bq_d = nc.declare_dram_parameter("W1bq", [128, 128], f16, False)
    W2d_d = nc.declare_dram_parameter("W2d", [128, 128], f16, False)
    out_d = nc.declare_dram_parameter("out", [128, PP], f16, True)

    with tile.TileContext(nc) as tc:
        frees = []

        def T(shape, dtype, name):
            t, f = tc.tile(shape, dtype, name=name)
            frees.append(f)
            return t

        W1bq_sb = T([128, 128], f16, "W1bq_sb")
        W2d_sb = T([128, 128], f16, "W2d_sb")
        ident = T([128, 128], f16, "ident")
        out_sb = T([128, PP], f16, "out_sb")

        nc.sync.dma_start(W1bq_sb[:, :], W1bq_d[:, :])
        nc.sync.dma_start(W2d_sb[:, :], W2d_d[:, :])
        make_identity(nc, ident[:, :])

        # Warm the PE clock (HAM needs ~3.4us of sustained activity) with a
        # short dummy-matmul burst that drains before the first relb DMA
        # lands, so the real pipeline starts at 2.4 GHz instead of 1.2.
        wrhs = T([128, G], f16, "wrhs")
        nc.vector.memset(wrhs[:, :], 0.0)
        with tc.tile_pool(name="warmpool", bufs=1, space="PSUM") as wmpl:
            scratch = wmpl.tile([128, G], f32, name="scratch")
            for _ in range(8):
                nc.tensor.matmul(scratch[:, :], lhsT=W2d_sb[:, :],
                                 rhs=wrhs[:, :], start=True, stop=True)

        with tc.tile_pool(name="relbpool", bufs=2) as rbpl, \
             tc.tile_pool(name="xgpool", bufs=4) as xgpl, \
             tc.tile_pool(name="upool", bufs=3, space="PSUM") as upl, \
             tc.tile_pool(name="wpool", bufs=3, space="PSUM") as wpl, \
             tc.tile_pool(name="accpool", bufs=1, space="PSUM") as accpl, \
             tc.tile_pool(name="rspool", bufs=4) as rspl, \
             tc.tile_pool(name="tpool", bufs=5) as tpl:

            us, rss, ws, ts, accs = {}, {}, {}, {}, {}
            xgs = {}

            def relb_load(q):
                relb_t = rbpl.tile([128, 8 * PCH], f16, name="relb")
                for r in range(4):
                    nc.sync.dma_start(relb_t[32 * r:32 * r + 8, :],
                                      relb_d[q, r, :, :])
                return relb_t

            relbs = {0: relb_load(0)}

            def xg_load(blk):
                xg_t = xgpl.tile([128, XGB * G], f16, name="xg")
                nc.sync.dma_start(xg_t[:, :],
                                  xg2_d[:, blk * XGB * G:(blk + 1) * XGB * G])
                return xg_t

            xgs[0] = xg_load(0)

            # 5-stage software pipeline over 256 groups:
            # S0 mm1(g) | S1 prelu(g-1) | S2 mm2(g-2) | S3 mult(g-3) | S4 mm3(g-4)
            for s in range(NG + 4):
                g = s
                if g < NG:
                    q, gc = g // GPC, g % GPC
                    k, i = gc // 2, gc % 2
                    r = k // 8
                    if g % XGB == 0 and g + XGB < NG:
                        xgs[g // XGB + 1] = xg_load(g // XGB + 1)
                    if gc == 0 and q + 1 < NCHUNK:
                        relbs[q + 1] = relb_load(q + 1)
                    if gc == 0:
                        accs[q] = accpl.tile([128, PCH], f32, name="acc")
                    u = upl.tile([128, G], f32, name="u")
                    us[g] = u
                    lo = (k % 8) * PCH + i * G
                    nc.tensor.matmul(u[:, :],
                                     lhsT=W1bq_sb[32 * r:32 * r + 8, :],
                                     rhs=relbs[q][32 * r:32 * r + 8,
                                                  lo:lo + G],
                                     start=True, stop=True,
                                     tile_position=(32 * r, 0))
                g1 = s - 1
                if 0 <= g1 < NG:
                    rs = rspl.tile([128, G], f16, name="rs")
                    rss[g1] = rs
                    nc.scalar.activation(rs[:, :], us.pop(g1)[:, :], Prelu,
                                         alpha=0.1)
                g2 = s - 2
                if 0 <= g2 < NG:
                    w = wpl.tile([128, G], f32, name="w")
                    ws[g2] = w
                    nc.tensor.matmul(w[:, :], lhsT=W2d_sb[:, :],
                                     rhs=rss.pop(g2)[:, :],
                                     start=True, stop=True)
                g3 = s - 3
                if 0 <= g3 < NG:
                    t = tpl.tile([128, G], f16, name="t")
                    ts[g3] = t
                    xg_t = xgs[g3 // XGB]
                    xlo = (g3 % XGB) * G
                    nc.vector.tensor_tensor(t[:, :], ws.pop(g3)[:, :],
                                            xg_t[:, xlo:xlo + G], mult)
                g4 = s - 4
                if 0 <= g4 < NG:
                    q4, gc4 = g4 // GPC, g4 % GPC
                    k4, i4 = gc4 // 2, gc4 % 2
                    nc.tensor.matmul(accs[q4][:, i4 * G:(i4 + 1) * G],
                                     lhsT=ident[:, :], rhs=ts.pop(g4)[:, :],
                                     start=(k4 == 0), stop=(k4 == K - 1))
                    if gc4 == GPC - 1:
                        nc.vector.tensor_copy(
                            out_sb[:, q4 * PCH:(q4 + 1) * PCH],
                            accs.pop(q4)[:, :])
                        nc.sync.dma_start(
                            out_d[:, q4 * PCH:(q4 + 1) * PCH],
                            out_sb[:, q4 * PCH:(q4 + 1) * PCH])
        for f in reversed(frees):
            f()

    import bass_rust
    bass_rust.move_matmul_waits_to_ldweights(nc.m)
    bass_rust.generate_event_semaphores(nc)
    mybir.codegen_inst_isa_subclasses(nc)
    return nc


def _get_nc():
    global _BUILT
    if _BUILT is None:
        _BUILT = _build()
    return _BUILT


def _prep_core(x16, pos, nidx, c, W1bq, W2d):
    b, hh = c // 2, c % 2
    sl = slice(hh * HALF, (hh + 1) * HALF)
    idxh = nidx[b, sl]                                  # [HALF, K]
    xg = x16[b][idxh]                                   # [HALF, K, 64] f16
    rel = (pos[b, sl][:, None, :] - pos[b][idxh]).astype(np.float16)

    xgA = xg[0::2].reshape(NCHUNK, PCH, K, D).transpose(0, 2, 1, 3)
    xgB = xg[1::2].reshape(NCHUNK, PCH, K, D).transpose(0, 2, 1, 3)
    xg2 = np.empty((128, M2), np.float16)
    xg2[0:64] = xgA.reshape(M2, D).T
    xg2[64:128] = xgB.reshape(M2, D).T

    rb = np.empty((8, M2), np.float16)
    relA = rel[0::2].reshape(NCHUNK, PCH, K, 3).transpose(0, 2, 1, 3)
    relB = rel[1::2].reshape(NCHUNK, PCH, K, 3).transpose(0, 2, 1, 3)
    rb[0:3] = relA.reshape(M2, 3).T
    rb[3] = 1.0
    rb[4:7] = relB.reshape(M2, 3).T
    rb[7] = 1.0
    relb = np.ascontiguousarray(
        rb.reshape(8, NCHUNK, 4, 8 * PCH).transpose(1, 2, 0, 3))
    return dict(xg2=np.ascontiguousarray(xg2), relb=relb,
                W1bq=W1bq, W2d=W2d)


def kernel(x, pos, neighbor_idx, W1, b1, W2, b2):
    nc = _get_nc()
    W1b = np.vstack([W1, b1[None, :]]).astype(np.float16)   # [4, 64]
    W1bq = np.zeros((128, 128), np.float16)
    for r in range(4):
        W1bq[32 * r:32 * r + 4, 0:64] = W1b
        W1bq[32 * r + 4:32 * r + 8, 64:128] = W1b
    W2d = np.zeros((128, 128), np.float16)
    W2f = W2.astype(np.float16)
    W2d[0:64, 0:64] = W2f
    W2d[64:128, 64:128] = W2f

    x16 = x.astype(np.float16)
    in_maps = [_prep_core(x16, pos, neighbor_idx, c, W1bq, W2d)
               for c in range(8)]
    global LAST_RESULTS
    res = bass_utils.run_bass_kernel_spmd(nc, in_maps, list(range(8)),
                                          trace=TRACE)
    LAST_RESULTS = res
    out = np.empty((B, N, D), np.float32)
    for c in range(8):
        b, hh = c // 2, c % 2
        r = np.asarray(res.results[c]["out"])               # [128, PP] f16
        half = r.T.reshape(PP, 2, D).reshape(HALF, D)
        out[b, hh * HALF:(hh + 1) * HALF] = half.astype(np.float32)
    if np.any(b2):
        for b in range(B):
            s = x[b][neighbor_idx[b]].sum(axis=1)
            out[b] += b2[None, :] * s
    return out
